# revision 1
# baseline (speedup 1.0000x reference)
"""Trainium2 Bass kernel for InterpBaselineEncoder (histogram binning).

Reference computation (per batch b of B=4):
  - coarsen 128x128 grid by 4x4 -> 32x32=1024 cells (grid_loc = regular
    meshgrid centers, grid_val = 4x4 mean of yc_on_grid)
  - bin U=8192 off-grid points to L1-nearest cell; scatter-mean yc_off
    values + the on-grid cell value into each cell
  - bin T=4096 target points the same way and gather the cell averages

Because xc_on_grid is a regular meshgrid (linspace(0,1,128) pooled 4x4),
the L1 argmin factorizes into independent row/col bins with closed form
clamp(floor(p*inv + off + 0.5), 0, 31).  The scatter becomes a one-hot
matmul: with i=row, j=col split as j = 16*jh + jl, accumulate
  psum[(2i+jh), (jl, y')] += onehot64(2i+jh)[u] * (onehot16(jl)[u] * y'[u])
over points u, where y' = [y, 1] (9 wide; the ones column yields counts).
The 1024 on-grid cell values enter as pseudo-points, which realizes the
reference's (sums + grid_val) / (counts + 1) for free.  The target gather
is a one-hot matmul over (2i+jh) plus an elementwise jl-contraction.

Sharding: 8 cores = 4 batches x 2 target halves (scatter duplicated per
pair, gather split).  SPMD: one Bass program, per-core input maps.
"""
import sys
import numpy as np

for _p in ("/opt/trn_rl_repo", "/opt/pypackages"):
    if _p not in sys.path:
        sys.path.insert(0, _p)

import ml_dtypes  # noqa: E402
from concourse import bass, bacc, mybir, tile  # noqa: E402
from concourse.bass_utils import run_bass_kernel_spmd  # noqa: E402

F32 = mybir.dt.float32
BF16 = mybir.dt.bfloat16
ALU = mybir.AluOpType

B, U, T, Y = 4, 8192, 4096, 8
GI = GJ = 32           # coarse grid 32x32
TH = T // 2            # targets per core (2048)
KT = U // 128          # 64 point tiles
NT = TH // 128         # 16 target tiles

# closed-form bin constants: centers c_k = (4k+1.5)/127, step 4/127
_C0 = 1.5 / 127.0
_INV = 127.0 / 4.0
_OFF0 = float(np.float32(-_C0 * _INV))
_MAGIC = 8388608.0  # 2^23: (z + M) - M rounds z to nearest-even integer

# packed f32 constant block layout [128, 257]
_CF_COLS = 257
# packed f32 input block layout [128, 672]
_IN_COLS = KT + KT + KT * Y + NT + NT


def _emit_bin(nc, pool, p_ap, n, nm):
    """clamp(round_ne(p*INV+OFF0), 0, 31) -> [128, n] f32 (3 vector ops)."""
    z = pool.tile([128, n], F32, tag=f"binz{nm}")
    idx = pool.tile([128, n], F32, tag=f"bini{nm}")
    nc.vector.tensor_scalar(z[:], p_ap, _INV, _OFF0, ALU.mult, ALU.add)
    nc.vector.tensor_scalar(idx[:], z[:], _MAGIC, _MAGIC, ALU.add, ALU.subtract)
    out = pool.tile([128, n], F32, tag=f"binc{nm}")
    nc.vector.tensor_scalar(out[:], idx[:], 0.0, 31.0, ALU.max, ALU.min)
    return out


def _emit_split(nc, pool, iv, jv, n, nm):
    """From i,j bins compute ihj = 2*i + j//16 and jl = j%16."""
    jh = pool.tile([128, n], F32, tag=f"jh{nm}")
    jh16 = pool.tile([128, n], F32, tag=f"jh16{nm}")
    jl = pool.tile([128, n], F32, tag=f"jl{nm}")
    i2 = pool.tile([128, n], F32, tag=f"i2{nm}")
    ihj = pool.tile([128, n], F32, tag=f"ihj{nm}")
    nc.vector.tensor_scalar(jh[:], jv[:], 16.0, None, ALU.is_ge)
    nc.vector.tensor_scalar(jh16[:], jh[:], 16.0, None, ALU.mult)
    nc.vector.tensor_tensor(jl[:], jv[:], jh16[:], ALU.subtract)
    nc.vector.tensor_scalar(i2[:], iv[:], 2.0, None, ALU.mult)
    nc.vector.tensor_tensor(ihj[:], i2[:], jh[:], ALU.add)
    return ihj, jl


def build_nc(loop_n=0):
    nc = bacc.Bacc("TRN2", target_bir_lowering=False, debug=False)

    constF = nc.declare_dram_parameter("constF", [128, _CF_COLS], F32,
                                       isOutput=False)
    selC = nc.declare_dram_parameter("selC", [16, NT * 64], BF16,
                                     isOutput=False)
    inF = nc.declare_dram_parameter("inF", [128, _IN_COLS], F32,
                                    isOutput=False)
    ycON = nc.declare_dram_parameter("ycON", [128, 1024], F32, isOutput=False)
    out_d = nc.declare_dram_parameter("out", [TH, Y], F32, isOutput=True)

    with tile.TileContext(nc) as tc:
        with (
            tc.tile_pool(name="const", bufs=1) as cpool,
            tc.tile_pool(name="work", bufs=1) as wpool,
            tc.tile_pool(name="psS", bufs=1, space="PSUM") as psS,
            tc.tile_pool(name="psP", bufs=1, space="PSUM") as psP,
            tc.tile_pool(name="psB", bufs=2, space="PSUM") as psB,
            tc.tile_pool(name="psR", bufs=2, space="PSUM") as psR,
        ):
            import contextlib
            loop_ctx = tc.For_i(0, loop_n, 1) if loop_n else contextlib.nullcontext()
            with loop_ctx:
                cf = cpool.tile([128, _CF_COLS], F32, tag="cf")
                nc.sync.dma_start(cf[:], constF[:])
                c_selC = cpool.tile([16, NT * 64], BF16, tag="selC")
                nc.sync.dma_start(c_selC[:], selC[:])
                tin = wpool.tile([128, _IN_COLS], F32, tag="tin")
                nc.sync.dma_start(tin[:], inF[:])
                t_ycon = wpool.tile([128, 1024], F32, tag="ycon")
                nc.sync.dma_start(t_ycon[:], ycON[:])

                c_iota64 = cf[:, 0:64]
                c_iota16 = cf[:, 64:80]
                c_ident = cf[:, 80:208]
                c_pmat = cf[:, 208:240]
                c_iotaP64 = cf[0:64, 240:241]
                c_ihjps = cf[:, 241:249]
                c_jlps = cf[:, 249:257]
                c_sel = c_selC[:].rearrange("p (n m) -> p n m", m=64)

                o = 0
                t_py = tin[:, o:o + KT]; o += KT
                t_px = tin[:, o:o + KT]; o += KT
                t_yoff = tin[:, o:o + KT * Y].rearrange("p (k y) -> p k y", y=Y)
                o += KT * Y
                t_xty = tin[:, o:o + NT]; o += NT
                t_xtx = tin[:, o:o + NT]; o += NT

                # ---- off-grid binning ----
                ioff = _emit_bin(nc, wpool, t_py, KT, "o")
                joff = _emit_bin(nc, wpool, t_px, KT, "o2")
                ihj, jl = _emit_split(nc, wpool, ioff, joff, KT, "o")

                # ---- one-hots + W2, chunked for overlap ----
                ra = wpool.tile([128, KT, 64], BF16, tag="ra")
                bl = wpool.tile([128, KT, 16], BF16, tag="bl")
                ybf = wpool.tile([128, KT, 9], BF16, tag="ybf")
                w2 = wpool.tile([128, KT, 16, 9], BF16, tag="w2")
                nc.vector.memset(ybf[:, :, 8:9], 1.0)
                CH = 16
                for c0 in range(0, KT, CH):
                    sl = slice(c0, c0 + CH)
                    nc.vector.tensor_tensor(
                        ra[:, sl, :],
                        c_iota64.unsqueeze(1).broadcast_to((128, CH, 64)),
                        ihj[:, sl].unsqueeze(2).broadcast_to((128, CH, 64)),
                        ALU.is_equal,
                    )
                    nc.vector.tensor_tensor(
                        bl[:, sl, :],
                        c_iota16.unsqueeze(1).broadcast_to((128, CH, 16)),
                        jl[:, sl].unsqueeze(2).broadcast_to((128, CH, 16)),
                        ALU.is_equal,
                    )
                    nc.scalar.copy(ybf[:, sl, 0:8], t_yoff[:, sl, :])
                    nc.vector.tensor_tensor(
                        w2[:, sl, :, :],
                        bl[:, sl, :].unsqueeze(3).broadcast_to((128, CH, 16, 9)),
                        ybf[:, sl, :].unsqueeze(2).broadcast_to((128, CH, 16, 9)),
                        ALU.mult,
                    )

                # ---- pooling of on-grid values -> pseudo-point values ----
                pp = psP.tile([32, 1024], F32, tag="pp")
                nc.tensor.matmul(pp[:, 0:512], c_pmat, t_ycon[:, 0:512],
                                 start=True, stop=True)
                nc.tensor.matmul(pp[:, 512:1024], c_pmat, t_ycon[:, 512:1024],
                                 start=True, stop=True)
                ppsb = wpool.tile([32, 1024], F32, tag="ppsb")
                nc.scalar.copy(ppsb[:], pp[:])
                ppv = ppsb[:].rearrange("p (j c y) -> p j c y", c=4, y=Y)
                tA = wpool.tile([32, GJ, Y], F32, tag="tA")
                tB = wpool.tile([32, GJ, Y], F32, tag="tB")
                gva = wpool.tile([32, GJ, 9], F32, tag="gva")
                nc.vector.tensor_tensor(tA[:], ppv[:, :, 0, :], ppv[:, :, 1, :], ALU.add)
                nc.vector.tensor_tensor(tB[:], ppv[:, :, 2, :], ppv[:, :, 3, :], ALU.add)
                nc.vector.tensor_tensor(gva[:, :, 0:8], tA[:], tB[:], ALU.add)
                nc.vector.memset(gva[:, :, 8:9], 1.0)
                gvabf = wpool.tile([32, GJ, 9], BF16, tag="gvabf")
                nc.scalar.copy(gvabf[:], gva[:])
                yps = wpool.tile([128, 8, 9], BF16, tag="yps")
                nc.sync.dma_start(yps[:], gvabf[:])

                raps = wpool.tile([128, 8, 64], BF16, tag="raps")
                blps = wpool.tile([128, 8, 16], BF16, tag="blps")
                w2ps = wpool.tile([128, 8, 16, 9], BF16, tag="w2ps")
                nc.vector.tensor_tensor(
                    raps[:],
                    c_iota64.unsqueeze(1).broadcast_to((128, 8, 64)),
                    c_ihjps.unsqueeze(2).broadcast_to((128, 8, 64)),
                    ALU.is_equal,
                )
                nc.vector.tensor_tensor(
                    blps[:],
                    c_iota16.unsqueeze(1).broadcast_to((128, 8, 16)),
                    c_jlps.unsqueeze(2).broadcast_to((128, 8, 16)),
                    ALU.is_equal,
                )
                nc.vector.tensor_tensor(
                    w2ps[:],
                    blps[:].unsqueeze(3).broadcast_to((128, 8, 16, 9)),
                    yps[:].unsqueeze(2).broadcast_to((128, 8, 16, 9)),
                    ALU.mult,
                )

                # ---- scatter matmuls: psum[64, 144] accumulates 72 tiles ----
                ps = psS.tile([64, 16 * 9], F32, tag="ps")
                for k in range(KT):
                    nc.tensor.matmul(ps[:], ra[:, k, :], w2[:, k, :, :],
                                     start=(k == 0), stop=False)
                for m in range(8):
                    nc.tensor.matmul(ps[:], raps[:, m, :], w2ps[:, m, :, :],
                                     start=False, stop=(m == 7))

                # ---- averages: avg[64, (y, jl)] bf16 ----
                psv = ps[:].rearrange("p (j y) -> p j y", y=9)
                rc = wpool.tile([64, 16], F32, tag="rc")
                nc.vector.reciprocal(rc[:], psv[:, :, 8])
                avg = wpool.tile([64, Y, 16], BF16, tag="avg")
                nc.vector.tensor_tensor(
                    avg[:],
                    psv[:, :, 0:8].transpose([0, 2, 1]),
                    rc[:].unsqueeze(1).broadcast_to((64, Y, 16)),
                    ALU.mult,
                )

                # ---- target binning + gather ----
                it = _emit_bin(nc, wpool, t_xty, NT, "t")
                jt = _emit_bin(nc, wpool, t_xtx, NT, "t2")
                ihjt, jlt = _emit_split(nc, wpool, it, jt, NT, "t")

                pst = psP.tile([16, 128], F32, tag="pp")  # reuse slot
                nc.tensor.transpose(pst[:], ihjt[:], c_ident)
                ihjTbf = wpool.tile([16, 128], BF16, tag="ihjTbf")
                nc.scalar.copy(ihjTbf[:], pst[:])

                zttl = wpool.tile([128, NT, 16], F32, tag="zttl")
                nc.vector.tensor_tensor(
                    zttl[:],
                    c_iota16.unsqueeze(1).broadcast_to((128, NT, 16)),
                    jlt[:].unsqueeze(2).broadcast_to((128, NT, 16)),
                    ALU.is_equal,
                )

                outsb = wpool.tile([128, NT, Y], F32, tag="outsb")
                for n in range(NT):
                    pb = psB.tile([64, 128], F32, tag="pb")
                    nc.tensor.matmul(pb[:], c_sel[:, n, :], ihjTbf[:],
                                     start=True, stop=True)
                    rt2 = wpool.tile([64, 128], BF16, tag="rt2")
                    nc.vector.tensor_scalar(rt2[:], pb[:], c_iotaP64, None,
                                            ALU.is_equal)
                    rv = psR.tile([128, 128], F32, tag="rv")
                    nc.tensor.matmul(rv[:], rt2[:], avg[:].rearrange("p y j -> p (y j)"),
                                     start=True, stop=True)
                    tmp = wpool.tile([128, Y, 16], F32, tag="tmp")
                    nc.vector.tensor_tensor(
                        tmp[:],
                        rv[:].rearrange("p (y j) -> p y j", j=16),
                        zttl[:, n, :].unsqueeze(1).broadcast_to((128, Y, 16)),
                        ALU.mult,
                    )
                    nc.vector.tensor_reduce(outsb[:, n, :], tmp[:],
                                            axis=mybir.AxisListType.X, op=ALU.add)

                nc.sync.dma_start(
                    out_d[:].rearrange("(n p) y -> p n y", p=128), outsb[:]
                )
    nc.compile()
    return nc


def _consts():
    pvals = np.zeros((128, 32), np.float32)
    for h in range(128):
        pvals[h, h // 4] = 1.0 / 16.0
    s = 8 * np.arange(128)[:, None] + np.arange(8)[None, :]  # [128, 8]
    si, sj = s // 32, s % 32
    cf = np.zeros((128, _CF_COLS), np.float32)
    cf[:, 0:64] = np.arange(64, dtype=np.float32)[None, :]
    cf[:, 64:80] = np.arange(16, dtype=np.float32)[None, :]
    cf[:, 80:208] = np.eye(128, dtype=np.float32)
    cf[:, 208:240] = pvals
    cf[:, 240] = np.arange(128, dtype=np.float32)
    cf[:, 241:249] = (2 * si + sj // 16).astype(np.float32)
    cf[:, 249:257] = (sj % 16).astype(np.float32)
    sel = np.eye(16, dtype=np.float32)[:, :, None].repeat(64, axis=2)
    return {
        "constF": cf,
        "selC": np.ascontiguousarray(sel.reshape(16, NT * 64)).astype(
            ml_dtypes.bfloat16),
    }


def _stage_core(xc_off, yc_off, yc_on, xt, b, half):
    m = {}
    fin = np.empty((128, _IN_COLS), np.float32)
    o = 0
    fin[:, o:o + KT] = xc_off[b, :, 0].reshape(KT, 128).T; o += KT
    fin[:, o:o + KT] = xc_off[b, :, 1].reshape(KT, 128).T; o += KT
    fin[:, o:o + KT * Y] = yc_off[b].reshape(KT, 128, Y).transpose(1, 0, 2) \
        .reshape(128, KT * Y); o += KT * Y
    sl = slice(half * TH, (half + 1) * TH)
    fin[:, o:o + NT] = xt[b, sl, 0].reshape(NT, 128).T; o += NT
    fin[:, o:o + NT] = xt[b, sl, 1].reshape(NT, 128).T; o += NT
    m["inF"] = fin
    m["ycON"] = np.ascontiguousarray(yc_on[b].reshape(128, 1024))
    return m


_NC = None


def kernel(xc_off_grid, yc_off_grid, xc_on_grid, yc_on_grid, xt):
    global _NC
    if _NC is None:
        _NC = build_nc()
    nc = _NC
    consts = _consts()

    xc_off_grid = np.ascontiguousarray(xc_off_grid, np.float32)
    yc_off_grid = np.ascontiguousarray(yc_off_grid, np.float32)
    yc_on_grid = np.ascontiguousarray(yc_on_grid, np.float32)
    xt = np.ascontiguousarray(xt, np.float32)

    in_maps = []
    for core in range(8):
        b, half = core // 2, core % 2
        m = dict(consts)
        m.update(_stage_core(xc_off_grid, yc_off_grid, yc_on_grid, xt, b, half))
        in_maps.append(m)

    res = run_bass_kernel_spmd(nc, in_maps, list(range(8)))
    out = np.empty((B, T, Y), np.float32)
    for core in range(8):
        b, half = core // 2, core % 2
        out[b, half * TH:(half + 1) * TH] = res.results[core]["out"]
    return out

